# revision 1
# baseline (speedup 1.0000x reference)
"""Trainium2 Bass kernel for nn_BilinearUpsampler.

out[b,c,i,j] = sum_k softmax_k(MLP(poseMap)[c,k,i,j]) * xpad[b,c,Y[i,j]+dy_k,X[i,j]+dx_k]

Strategy (8 NeuronCores, output-pixel sharded, 32768 px/core):
  - pixels-on-partitions layout: every on-chip tensor is [128 pixels, ...free]
  - MLP (3->64->256->576 1x1 convs) on PE; final matmul flipped (lhsT = h2
    tile) so logits land as [128 px, 576] in PSUM; bias b3 added via a K=1
    ones-row matmul; exp eviction on ACT -> e [128, 576] bf16
  - 3x3 patch gather via gpsimd.dma_gather from DRAM: x stored as
    [130*130 cells, 128 (b,c)] bf16; one 768B descriptor per (pixel, dy)
    with elem_step=128 elems (256B) giving overlapping 3-cell windows
  - 9-tap weighted sum + softmax denominator + normalize on DVE (bf16 2x)
  - per-core output [32768 px, 128 (b,c)] f32; host reassembles layout
"""

import sys
import os

sys.path.insert(0, "/opt/trn_rl_repo")

import numpy as np
import ml_dtypes

import concourse.bass as bass
import concourse.bacc as bacc
import concourse.mybir as mybir
import concourse.tile as tile
from concourse.bass_utils import run_bass_kernel_spmd
import bass_rust

BF16 = mybir.dt.bfloat16
F32 = mybir.dt.float32
I16 = mybir.dt.int16
AF = mybir.ActivationFunctionType

NCORES = 8
C = 64
KS = 3
BS = 2
HI = WI = 128
HO = WO = 512
HP = HI + 2  # 130 padded
NCELL = HP * HP  # 16900
NWIN = NCELL - 2  # gatherable 3-cell windows
PXTOT = HO * WO
PX = PXTOT // NCORES  # 32768 pixels per core

TT = 1024  # pixel tile
SUB = TT // 128  # 8 subtiles of 128 px
NT = PX // TT  # 32 tiles

LAST_RESULT = None  # BassKernelResults of the most recent run (for test.py)

_PROG_CACHE = {}


def build_program(px=PX, tt=TT):
    sub = tt // 128
    nt = px // tt
    nc = bacc.Bacc("TRN2", target_bir_lowering=False, debug=False,
                   num_devices=NCORES)

    xw_d = nc.dram_tensor("xw", [NCELL * 128], BF16, kind="ExternalInput")
    idx_d = nc.dram_tensor("idxw", [3, 128, px // 16], I16, kind="ExternalInput")
    pose_d = nc.dram_tensor("pose", [3, px], BF16, kind="ExternalInput")
    w1t_d = nc.dram_tensor("w1t", [3, 64], BF16, kind="ExternalInput")
    w2t_d = nc.dram_tensor("w2t", [64, 256], BF16, kind="ExternalInput")
    w3km_d = nc.dram_tensor("w3km", [256, 576], BF16, kind="ExternalInput")
    b3km_d = nc.dram_tensor("b3km", [1, 576], BF16, kind="ExternalInput")
    b1_d = nc.dram_tensor("b1", [64, 1], F32, kind="ExternalInput")
    b2_d = nc.dram_tensor("b2", [256, 1], F32, kind="ExternalInput")
    out_d = nc.dram_tensor("out", [px // 128, 128, 128], BF16, kind="ExternalOutput")

    # overlapping 3-cell window view of x: [NWIN, 384] with row stride 128
    def x_windows_ap():
        ap = xw_d[:].copy()
        ap.ap = bass_rust.VecI64Pair([(128, NWIN), (1, 384)])
        return ap

    with tile.TileContext(nc) as tc:
        with (
            tc.tile_pool(name="consts", bufs=1) as cpool,
            tc.tile_pool(name="mlp", bufs=2) as mpool,
            tc.tile_pool(name="gath", bufs=3) as gpool,
            tc.tile_pool(name="ework", bufs=3) as epool,
            tc.tile_pool(name="dve", bufs=1) as vpool,
            tc.tile_pool(name="outp", bufs=3) as opool,
            tc.tile_pool(name="ph1", bufs=1, space="PSUM") as ph1,
            tc.tile_pool(name="ph2", bufs=1, space="PSUM") as ph2,
            tc.tile_pool(name="pw", bufs=3, space="PSUM") as pw,
        ):
            # ---- constants ----
            w1t = cpool.tile([3, 64], BF16, tag="w1t")
            nc.sync.dma_start(w1t[:], w1t_d[:])
            w2t = cpool.tile([64, 256], BF16, tag="w2t")
            nc.sync.dma_start(w2t[:], w2t_d[:])
            w3km0 = cpool.tile([128, 576], BF16, tag="w3km0")
            nc.sync.dma_start(w3km0[:], w3km_d[0:128])
            w3km1 = cpool.tile([128, 576], BF16, tag="w3km1")
            nc.sync.dma_start(w3km1[:], w3km_d[128:256])
            b3km = cpool.tile([1, 576], BF16, tag="b3km")
            nc.sync.dma_start(b3km[:], b3km_d[:])
            b1t = cpool.tile([64, 1], F32, tag="b1t")
            nc.sync.dma_start(b1t[:], b1_d[:])
            b2t0 = cpool.tile([128, 1], F32, tag="b2t0")
            nc.sync.dma_start(b2t0[:], b2_d[0:128])
            b2t1 = cpool.tile([128, 1], F32, tag="b2t1")
            nc.sync.dma_start(b2t1[:], b2_d[128:256])
            ones = cpool.tile([1, 128], BF16, tag="ones")
            nc.vector.memset(ones[:], 1.0)
            idxt = cpool.tile([128, 3, px // 16], I16, tag="idxt")
            for dy in range(3):
                nc.sync.dma_start(idxt[:, dy, :], idx_d[dy])

            xwin = x_windows_ap()

            sched = [(i * tt, tt) for i in range(nt)]

            for t0, tti in sched:
                # ---- MLP stage ----
                sub_i = tti // 128
                p3 = mpool.tile([3, tti], BF16, tag="p3")
                nc.sync.dma_start(p3[:], pose_d[:, t0:t0 + tti])
                h1s = mpool.tile([64, tti], BF16, tag="h1s")
                h2s0 = mpool.tile([128, tti], BF16, tag="h2s0")
                h2s1 = mpool.tile([128, tti], BF16, tag="h2s1")
                for q in range(tti // 512):
                    qs = slice(q * 512, (q + 1) * 512)
                    h1p = ph1.tile([64, 512], F32, tag="h1p")
                    nc.tensor.matmul(h1p[:], w1t[:], p3[:, qs],
                                     start=True, stop=True)
                    nc.scalar.activation(h1s[:, qs], h1p[:], AF.Relu,
                                         bias=b1t[:])
                    for cc, (h2s, b2t) in ((0, (h2s0, b2t0)), (1, (h2s1, b2t1))):
                        h2p = ph2.tile([128, 512], F32, tag="h2p")
                        nc.tensor.matmul(h2p[:], w2t[:, cc * 128:(cc + 1) * 128],
                                         h1s[:, qs], start=True, stop=True)
                        nc.scalar.activation(h2s[:, qs], h2p[:], AF.Relu,
                                             bias=b2t[:])

                # ---- gather stage ----
                g = gpool.tile([128, 3, sub_i, 384], BF16, tag="g")
                for dy in range(3):
                    nc.gpsimd.dma_gather(
                        out_ap=g[:, dy, :, :],
                        in_ap=xwin,
                        idxs_ap=idxt[:, dy, t0 // 16:(t0 + tti) // 16],
                        num_idxs=tti,
                        num_idxs_reg=tti,
                        elem_size=384,
                        elem_step=128,
                    )

                # ---- logits + exp ----
                e_t = epool.tile([128, sub_i, 576], BF16, tag="e_t")
                for s in range(sub_i):
                    ss = slice(s * 128, s * 128 + 128)
                    wp = pw.tile([128, 576], F32, tag="wp")
                    for r0, r1 in ((0, 512), (512, 576)):
                        nc.tensor.matmul(wp[:, r0:r1], h2s0[:, ss],
                                         w3km0[:, r0:r1], start=True, stop=False)
                        nc.tensor.matmul(wp[:, r0:r1], h2s1[:, ss],
                                         w3km1[:, r0:r1], start=False, stop=False)
                        nc.tensor.matmul(wp[:, r0:r1], ones[:],
                                         b3km[:, r0:r1], start=False, stop=True)
                    nc.scalar.activation(e_t[:, s, :], wp[:], AF.Exp)

                # ---- taps on DVE ----
                # one mul per (dy, b): iter (s, dx, c); e needs no broadcast
                prods = vpool.tile([128, 9, sub_i, 128], BF16, tag="prods")
                pr4 = prods[:].rearrange("p k s (b c) -> p k s b c", b=2)
                g5 = g[:].rearrange("p d s (x b c) -> p d s x b c", x=3, b=2)
                for dy in range(3):
                    for b in range(2):
                        # out: k=3dy+dx, fixed b -> iter (s, dx, c)
                        o = pr4[:, 3 * dy:3 * dy + 3, :, b, :].rearrange(
                            "p x s c -> p s x c")
                        gk = g5[:, dy, :, :, b, :]          # [128, s, x, c]
                        ek = e_t[:, :, 3 * dy * 64:(3 * dy + 3) * 64].rearrange(
                            "p s (x c) -> p s x c", x=3)
                        nc.vector.tensor_mul(o, gk, ek)
                q1 = vpool.tile([128, 4, sub_i, 128], BF16, tag="q1")
                nc.vector.tensor_add(q1[:], prods[:, 0:4], prods[:, 4:8])
                q2 = vpool.tile([128, 2, sub_i, 128], BF16, tag="q2")
                nc.vector.tensor_add(q2[:], q1[:, 0:2], q1[:, 2:4])
                acc = vpool.tile([128, sub_i, 128], BF16, tag="acc")
                nc.vector.tensor_add(acc[:], q2[:, 0], q2[:, 1])
                acc2 = vpool.tile([128, sub_i, 128], BF16, tag="acc2")
                nc.vector.tensor_add(acc2[:], acc[:], prods[:, 8])

                # ---- softmax denominator ----
                d1 = vpool.tile([128, sub_i, 256], BF16, tag="d1")
                nc.vector.tensor_add(d1[:], e_t[:, :, 0:256], e_t[:, :, 256:512])
                d2 = vpool.tile([128, sub_i, 128], F32, tag="d2")
                nc.gpsimd.tensor_add(d2[:], d1[:, :, 0:128], d1[:, :, 128:256])
                d3 = vpool.tile([128, sub_i, 64], F32, tag="d3")
                nc.gpsimd.tensor_add(d3[:], d2[:, :, 0:64], d2[:, :, 64:128])
                den = vpool.tile([128, sub_i, 64], F32, tag="den")
                nc.gpsimd.tensor_add(den[:], d3[:], e_t[:, :, 512:576])
                rden = vpool.tile([128, sub_i, 64], F32, tag="rden")
                nc.vector.reciprocal(rden[:], den[:])

                # ---- normalize + store ----
                out_t = opool.tile([128, sub_i, 128], BF16, tag="out_t")
                ov = out_t[:].rearrange("p s (b c) -> p s b c", b=2)
                av = acc2[:].rearrange("p s (b c) -> p s b c", b=2)
                rv = rden[:].unsqueeze(2).broadcast_to((128, sub_i, 2, 64))
                nc.gpsimd.tensor_mul(ov, av, rv)
                dview = out_d[t0 // 128:(t0 + tti) // 128].rearrange("s p b -> p s b")
                nc.sync.dma_start(dview, out_t[:])

    nc.compile()
    return nc


def _host_prep(x, poseMap, W1, b1, W2, b2, W3, b3, interMapY, interMapX,
               px=PX, tt=TT):  # noqa: C901
    bf = ml_dtypes.bfloat16
    xp = np.pad(np.asarray(x, np.float32), ((0, 0), (0, 0), (1, 1), (1, 1)))
    # [cell, (b,c)] with c minor
    xw = np.ascontiguousarray(np.transpose(xp, (2, 3, 0, 1))).reshape(NCELL * 128)
    xw = xw.astype(bf)

    Y = np.asarray(interMapY).astype(np.int64).reshape(-1)
    X = np.asarray(interMapX).astype(np.int64).reshape(-1)
    m = ((Y + 0) * HP + X).astype(np.int32)  # base window (padded coords)

    pose = np.asarray(poseMap, np.float32)[0].reshape(3, PXTOT)

    w1t = np.ascontiguousarray(np.asarray(W1, np.float32).T).astype(bf)  # [3,64]
    w2t = np.ascontiguousarray(np.asarray(W2, np.float32).T).astype(bf)  # [64,256]
    W3r = np.asarray(W3, np.float32).reshape(C, 9, 256)
    w3km = np.ascontiguousarray(np.transpose(W3r, (2, 1, 0))).reshape(256, 576)
    w3km = w3km.astype(bf)
    b3km = np.ascontiguousarray(
        np.asarray(b3, np.float32).reshape(C, 9).T).reshape(1, 576).astype(bf)
    b1c = np.asarray(b1, np.float32).reshape(64, 1)
    b2c = np.asarray(b2, np.float32).reshape(256, 1)

    in_maps = []
    for core in range(NCORES):
        sl = slice(core * px, (core + 1) * px)
        mc = m[sl]
        idxw = np.empty((3, 128, px // 16), np.int16)
        for dy in range(3):
            a = (mc + HP * dy).astype(np.int16).reshape(px // 16, 16)
            idxw[dy] = np.tile(a.T, (8, 1))
        in_maps.append({
            "xw": xw,
            "idxw": idxw,
            "pose": np.ascontiguousarray(pose[:, sl]).astype(bf),
            "w1t": w1t, "w2t": w2t, "w3km": w3km, "b3km": b3km,
            "b1": b1c, "b2": b2c,
        })
    return in_maps


def kernel(**inputs):
    global LAST_RESULT
    key = (PX, TT)
    if key not in _PROG_CACHE:
        _PROG_CACHE[key] = build_program(PX, TT)
    nc = _PROG_CACHE[key]
    in_maps = _host_prep(**inputs)
    os.environ.setdefault("BASS_NEVER_TRACE", "1")
    res = None
    last_err = None
    for attempt in range(3):
        try:
            res = run_bass_kernel_spmd(nc, in_maps, list(range(NCORES)))
            break
        except Exception as err:  # transient NRT_EXEC_UNIT_UNRECOVERABLE etc.
            last_err = err
            os.environ["NEURON_RT_RESET_CORES"] = "1"
    if res is None:
        raise last_err
    LAST_RESULT = res
    parts = [np.asarray(r["out"]).reshape(PX, 128) for r in res.results]
    full = np.concatenate(parts, axis=0).astype(np.float32)  # [PXTOT, 128]
    out = full.reshape(HO, WO, BS, C).transpose(2, 3, 0, 1)
    return np.ascontiguousarray(out)


if __name__ == "__main__":
    import json
    data = np.load(sys.argv[1] if len(sys.argv) > 1 else "work/inputs.npz")
    out = kernel(**{k: data[k] for k in data.files})
    print("out", out.shape, out.dtype, float(np.abs(out).max()))



# revision 45
# speedup vs baseline: 1.0533x; 1.0533x over previous
"""Trainium2 Bass kernel for nn_BilinearUpsampler.

out[b,c,i,j] = sum_k softmax_k(MLP(poseMap)[c,k,i,j]) * xpad[b,c,Y[i,j]+dy_k,X[i,j]+dx_k]

Strategy (8 NeuronCores, output-pixel sharded, 32768 px/core):
  - pixels-on-partitions layout: every on-chip tensor is [128 pixels, ...free]
  - MLP (3->64->256->576 1x1 convs) on PE; final matmul flipped (lhsT = h2
    tile) so logits land as [128 px, 576] in PSUM; bias b3 added via a K=1
    ones-row matmul; exp eviction on ACT -> e [128, 576] bf16
  - 3x3 patch gather via ONE gpsimd.dma_gather descriptor per pixel:
    x stored as "supercells" [128 y, 130 x] each holding 3 stacked rows
    (y..y+2) x 128 (b,c) bf16 = 768B; a 3-supercell window (2304B) covers
    the full 3x3 neighborhood; tap order (dx, dy)
  - 9-tap weighted sum + softmax denominator split across DVE/gpsimd (bf16 2x)
  - normalization via gpsimd tensor_tensor(divide) directly by the denominator
  - per-core output [128 p, 256 (t,s), 128 (b,c)] bf16, 2KB-contiguous
    per-partition DMA descriptors; host reassembles layout
"""

import sys
import os

sys.path.insert(0, "/opt/trn_rl_repo")

import numpy as np
import ml_dtypes

import concourse.bass as bass
import concourse.bacc as bacc
import concourse.mybir as mybir
import concourse.tile as tile
from concourse.bass_utils import run_bass_kernel_spmd
import bass_rust

BF16 = mybir.dt.bfloat16
F32 = mybir.dt.float32
I16 = mybir.dt.int16
AF = mybir.ActivationFunctionType
ALU = mybir.AluOpType

NCORES = 8
C = 64
KS = 3
BS = 2
HI = WI = 128
HO = WO = 512
HP = HI + 2  # 130 padded
SCW = HP  # supercell grid width (x: 0..129)
SCH = HI  # supercell grid height (y: 0..127); supercell y holds rows y..y+2
NSC = SCH * SCW  # 16640 supercells
NWIN = NSC - 2  # gatherable 3-supercell windows
PXTOT = HO * WO
PX = PXTOT // NCORES  # 32768 pixels per core

TT = 1024  # pixel tile
SUB = TT // 128  # 8 subtiles of 128 px
NT = PX // TT  # 32 tiles

LAST_RESULT = None  # BassKernelResults of the most recent run (for test.py)

_PROG_CACHE = {}


def build_program(px=PX, tt=TT):
    # schedule: small warmup tiles fill the pipeline quickly, then full tiles
    sched = []
    t0 = 0
    for tti in [tt] * (px // tt):
        sched.append((t0, tti))
        t0 += tti
    assert t0 == px
    nt = len(sched)
    nc = bacc.Bacc("TRN2", target_bir_lowering=False, debug=False,
                   num_devices=NCORES)

    xw_d = nc.dram_tensor("xw", [NSC * 384], BF16, kind="ExternalInput")
    idx_d = nc.dram_tensor("idxw", [128, px // 16], I16, kind="ExternalInput")
    # pose row 3 is all-ones (host-added); w1t/w2t carry the biases as an
    # extra input row against it, and w1t col 64 regenerates the ones row
    # for h1s (relu(1) = 1) so w2t's bias row has a const-1 partner too
    pose_d = nc.dram_tensor("pose", [4, px], BF16, kind="ExternalInput")
    w1t_d = nc.dram_tensor("w1t", [4, 65], BF16, kind="ExternalInput")
    w2t_d = nc.dram_tensor("w2t", [65, 256], BF16, kind="ExternalInput")
    w3km_d = nc.dram_tensor("w3km", [256, 576], BF16, kind="ExternalInput")
    b3km_d = nc.dram_tensor("b3km", [1, 576], BF16, kind="ExternalInput")
    out_d = nc.dram_tensor("out", [128, px // 128, 128], BF16,
                           kind="ExternalOutput")

    # overlapping 3-supercell window view of x: [NWIN, 1152] row stride 384
    def x_windows_ap():
        ap = xw_d[:].copy()
        ap.ap = bass_rust.VecI64Pair([(384, NWIN), (1, 1152)])
        return ap

    with tile.TileContext(nc) as tc:
        with (
            tc.tile_pool(name="consts", bufs=1) as cpool,
            tc.tile_pool(name="mlp", bufs=2) as mpool,
            tc.tile_pool(name="pose", bufs=4) as ppool,
            tc.tile_pool(name="gath", bufs=4) as gpool,
            tc.tile_pool(name="ework", bufs=4) as epool,
            tc.tile_pool(name="dve", bufs=1) as vpool,
            tc.tile_pool(name="dpool", bufs=2) as dpool,
            tc.tile_pool(name="outp", bufs=2) as opool,
            tc.tile_pool(name="ph1", bufs=1, space="PSUM") as ph1,
            tc.tile_pool(name="ph2", bufs=1, space="PSUM") as ph2,
            tc.tile_pool(name="pw", bufs=2, space="PSUM") as pw,
        ):
            # ---- constants ----
            w1t = cpool.tile([4, 65], BF16, tag="w1t")
            nc.sync.dma_start(w1t[:], w1t_d[:])
            w2t = cpool.tile([65, 256], BF16, tag="w2t")
            nc.sync.dma_start(w2t[:], w2t_d[:])
            w3km0 = cpool.tile([128, 576], BF16, tag="w3km0")
            nc.sync.dma_start(w3km0[:], w3km_d[0:128])
            w3km1 = cpool.tile([128, 576], BF16, tag="w3km1")
            nc.sync.dma_start(w3km1[:], w3km_d[128:256])
            b3km = cpool.tile([1, 576], BF16, tag="b3km")
            nc.sync.dma_start(b3km[:], b3km_d[:])
            ones = cpool.tile([1, 128], BF16, tag="ones")
            nc.vector.memset(ones[:], 1.0)
            idxt = cpool.tile([128, px // 16], I16, tag="idxt")
            nc.sync.dma_start(idxt[:], idx_d[:])

            xwin = x_windows_ap()

            # prefetch pose tiles and gathers ahead of use; gathers hog the
            # shared DMA engines (6.5us each), so pose loads go first
            GDEPTH = 3
            PDEPTH = 3
            gtiles = {}
            ptiles = {}

            def issue_p3(t):
                t0, tti = sched[t]
                p3 = ppool.tile([4, tt], BF16, tag="p3")
                nc.sync.dma_start(p3[:, 0:tti], pose_d[:, t0:t0 + tti])
                ptiles[t] = p3

            def issue_gather(t):
                t0, tti = sched[t]
                g = gpool.tile([128, tt // 128, 1152], BF16, tag="g")
                nc.gpsimd.dma_gather(
                    out_ap=g[:, 0:tti // 128, :],
                    in_ap=xwin,
                    idxs_ap=idxt[:, t0 // 16:(t0 + tti) // 16],
                    num_idxs=tti,
                    num_idxs_reg=tti,
                    elem_size=1152,
                    elem_step=384,
                )
                gtiles[t] = g[:, 0:tti // 128, :]

            for t in range(min(PDEPTH, nt)):
                issue_p3(t)
            for t in range(min(GDEPTH, nt)):
                issue_gather(t)

            pend = {}

            def finish(t):
                # recip/normalize/store for tile t, issued one tile late so
                # the den->recip->mul ping-pong never blocks the FIFO heads
                ft0, fsub, facc2, fden = pend.pop(t)
                rden_f = dpool.tile([128, tt // 128, 64], F32, tag="rden")
                rden = rden_f[:, 0:fsub, :]
                nc.vector.reciprocal(rden, fden)
                out_f = opool.tile([128, tt // 128, 128], BF16, tag="out_t")
                out_t = out_f[:, 0:fsub, :]
                ov = out_t.rearrange("p s (b c) -> p s b c", b=2)
                av = facc2.rearrange("p s (b c) -> p s b c", b=2)
                dv = rden.unsqueeze(2).broadcast_to((128, fsub, 2, 64))
                nc.gpsimd.tensor_mul(ov, av, dv)
                nc.sync.dma_start(out_d[:, ft0 // 128:ft0 // 128 + fsub], out_t)

            for t in range(nt):
                t0, tti = sched[t]
                sub = tti // 128
                # ---- MLP stage ----
                if t + PDEPTH < nt:
                    issue_p3(t + PDEPTH)
                p3 = ptiles.pop(t)
                h1s = mpool.tile([65, tt], BF16, tag="h1s")
                h2s = mpool.tile([128, 2, tt], BF16, tag="h2s")
                for q0 in range(0, tti, 512):
                    qn = min(512, tti - q0)
                    qs = slice(q0, q0 + qn)
                    h1p = ph1.tile([65, 512], F32, tag="h1p")
                    nc.tensor.matmul(h1p[:, 0:qn], w1t[:], p3[:, qs],
                                     start=True, stop=True)
                    nc.scalar.activation(h1s[:, qs], h1p[:, 0:qn], AF.Relu)
                    h2p = ph2.tile([128, 2, 512], F32, tag="h2p")
                    for cc in range(2):
                        nc.tensor.matmul(h2p[:, cc, 0:qn],
                                         w2t[:, cc * 128:(cc + 1) * 128],
                                         h1s[:, qs], start=True, stop=True)
                    nc.scalar.activation(h2s[:, :, qs], h2p[:, :, 0:qn], AF.Relu)

                # ---- gather (prefetched GDEPTH tiles ahead) ----
                if t + GDEPTH < nt:
                    issue_gather(t + GDEPTH)
                g = gtiles.pop(t)

                # ---- logits + exp ----  (w3km columns in (dx, dy, c) order)
                e_full = epool.tile([128, tt // 128, 576], BF16, tag="e_t")
                e_t = e_full[:, 0:sub, :]
                for s in range(sub):
                    ss = slice(s * 128, s * 128 + 128)
                    wp = pw.tile([128, 576], F32, tag="wp")
                    for r0, r1 in ((0, 512), (512, 576)):
                        nc.tensor.matmul(wp[:, r0:r1], h2s[:, 0, ss],
                                         w3km0[:, r0:r1], start=True, stop=False)
                        nc.tensor.matmul(wp[:, r0:r1], h2s[:, 1, ss],
                                         w3km1[:, r0:r1], start=False, stop=False)
                        nc.tensor.matmul(wp[:, r0:r1], ones[:],
                                         b3km[:, r0:r1], start=False, stop=True)
                    nc.scalar.activation(e_t[:, s, :], wp[:], AF.Exp)

                # ---- taps on DVE ----
                # g window layout per pixel: (dx, dy, b, c); e: (dx, dy, c)
                # prods[p, k=(dx*3+dy), s, (b,c)]; muls split over s-halves so
                # they start after half the exp work
                prods_f = vpool.tile([128, 9, tt // 128, 128], BF16, tag="prods")
                prods = prods_f[:, :, 0:sub, :]
                # k = (dx,dy) is stride-contiguous in g, e and prods, so
                # each per-b mul is a 3-free-dim AP (ISA limit)
                pr5 = prods.rearrange("p k s (b c) -> p k s b c", b=2)
                g5 = g.rearrange("p s (k b c) -> p k s b c", k=9, b=2)
                e5 = e_t.rearrange("p s (k c) -> p k s c", k=9)
                for b in range(2):
                    nc.vector.tensor_mul(pr5[:, :, :, b, :],
                                         g5[:, :, :, b, :], e5)
                q1_f = vpool.tile([128, 4, tt // 128, 128], BF16, tag="q1")
                q1 = q1_f[:, :, 0:sub, :]
                nc.vector.tensor_add(q1, prods[:, 0:4], prods[:, 4:8])
                q2_f = vpool.tile([128, 2, tt // 128, 128], BF16, tag="q2")
                q2 = q2_f[:, :, 0:sub, :]
                nc.vector.tensor_add(q2, q1[:, 0:2], q1[:, 2:4])
                acc_f = vpool.tile([128, tt // 128, 128], BF16, tag="acc")
                acc = acc_f[:, 0:sub, :]
                nc.vector.tensor_add(acc, q2[:, 0], q2[:, 1])
                acc2_f = dpool.tile([128, tt // 128, 128], BF16, tag="acc2")
                acc2 = acc2_f[:, 0:sub, :]
                nc.vector.tensor_add(acc2, acc, prods[:, 8])

                # ---- softmax denominator: tree split DVE / gpsimd ----
                # last tiles keep the whole den tree on DVE so the final
                # recip/norm chain never waits on the lagging Pool stream
                tail = t >= nt - 2
                DVS = 256 if tail else 104  # d1 cols on DVE; rest on gpsimd
                dadd = nc.vector.tensor_add if tail else nc.gpsimd.tensor_add
                d1_f = dpool.tile([128, tt // 128, 256], BF16, tag="d1")
                d1 = d1_f[:, 0:sub, :]
                nc.vector.tensor_add(d1[:, :, 0:DVS], e_t[:, :, 0:DVS],
                                     e_t[:, :, 256:256 + DVS])
                if DVS < 256:
                    nc.gpsimd.tensor_add(d1[:, :, DVS:256], e_t[:, :, DVS:256],
                                         e_t[:, :, 256 + DVS:512])
                d2_f = dpool.tile([128, tt // 128, 128], F32, tag="d2")
                d2 = d2_f[:, 0:sub, :]
                dadd(d2, d1[:, :, 0:128], d1[:, :, 128:256])
                d3_f = dpool.tile([128, tt // 128, 64], F32, tag="d3")
                d3 = d3_f[:, 0:sub, :]
                dadd(d3, d2[:, :, 0:64], d2[:, :, 64:128])
                den_f = dpool.tile([128, tt // 128, 64], F32, tag="den")
                den = den_f[:, 0:sub, :]
                dadd(den, d3, e_t[:, :, 512:576])

                # ---- normalize + store are DEFERRED one tile (see below)
                pend[t] = (t0, sub, acc2, den)
                if t - 1 in pend:
                    finish(t - 1)

            finish(nt - 1)

    nc.compile()
    return nc


def _host_prep(x, poseMap, W1, b1, W2, b2, W3, b3, interMapY, interMapX,
               px=PX, tt=TT):  # noqa: C901
    bf = ml_dtypes.bfloat16
    xp = np.pad(np.asarray(x, np.float32), ((0, 0), (0, 0), (1, 1), (1, 1)))
    # supercell (y, x): rows y..y+2 at padded col x; layout (dy, b, c)
    xs = np.ascontiguousarray(np.transpose(xp, (2, 3, 0, 1)))  # [130y,130x,2,64]
    sw = np.lib.stride_tricks.sliding_window_view(xs, 3, axis=0)
    # sw: [128, 130, 2, 64, 3] -> [y, x, dy, b, c]
    xsc = np.ascontiguousarray(np.transpose(sw, (0, 1, 4, 2, 3)))
    xsc = xsc.reshape(NSC * 384).astype(bf)

    Y = np.asarray(interMapY).astype(np.int64).reshape(-1)
    X = np.asarray(interMapX).astype(np.int64).reshape(-1)
    m = (Y * SCW + X).astype(np.int32)  # base supercell window

    pose = np.concatenate([np.asarray(poseMap, np.float32)[0].reshape(3, PXTOT),
                           np.ones((1, PXTOT), np.float32)], axis=0)

    w1t = np.zeros((4, 65), np.float32)
    w1t[0:3, 0:64] = np.asarray(W1, np.float32).T
    w1t[3, 0:64] = np.asarray(b1, np.float32)
    w1t[3, 64] = 1.0  # regenerates the const-1 row in h1s
    w1t = w1t.astype(bf)
    w2t = np.concatenate([np.asarray(W2, np.float32).T,
                          np.asarray(b2, np.float32)[None, :]], axis=0)
    w2t = np.ascontiguousarray(w2t).astype(bf)  # [65,256]
    # W3 columns ordered (dx, dy, c): col j = (dx*3+dy)*64 + c
    W3r = np.asarray(W3, np.float32).reshape(C, KS, KS, 256)  # [c, dy, dx, :]
    w3km = np.ascontiguousarray(np.transpose(W3r, (3, 2, 1, 0))).reshape(256, 576)
    w3km = w3km.astype(bf)
    b3r = np.asarray(b3, np.float32).reshape(C, KS, KS)
    b3km = np.ascontiguousarray(np.transpose(b3r, (2, 1, 0))).reshape(1, 576)
    b3km = b3km.astype(bf)

    in_maps = []
    for core in range(NCORES):
        sl = slice(core * px, (core + 1) * px)
        a = m[sl].astype(np.int16).reshape(px // 16, 16)
        idxw = np.ascontiguousarray(np.tile(a.T, (8, 1)))  # [128, px//16]
        in_maps.append({
            "xw": xsc,
            "idxw": idxw,
            "pose": np.ascontiguousarray(pose[:, sl]).astype(bf),
            "w1t": w1t, "w2t": w2t, "w3km": w3km, "b3km": b3km,
        })
    return in_maps


def kernel(**inputs):
    global LAST_RESULT
    key = (PX, TT)
    if key not in _PROG_CACHE:
        _PROG_CACHE[key] = build_program(PX, TT)
    nc = _PROG_CACHE[key]
    in_maps = _host_prep(**inputs)
    os.environ.setdefault("BASS_NEVER_TRACE", "1")
    res = None
    last_err = None
    for attempt in range(3):
        try:
            res = run_bass_kernel_spmd(nc, in_maps, list(range(NCORES)))
            break
        except Exception as err:  # transient NRT_EXEC_UNIT_UNRECOVERABLE etc.
            last_err = err
            os.environ["NEURON_RT_RESET_CORES"] = "1"
    if res is None:
        raise last_err
    LAST_RESULT = res
    parts = []
    for r in res.results:
        arr = np.asarray(r["out"]).reshape(128, PX // 128, 128)
        parts.append(np.transpose(arr, (1, 0, 2)).reshape(PX, 128))
    full = np.concatenate(parts, axis=0).astype(np.float32)  # [PXTOT, 128]
    out = full.reshape(HO, WO, BS, C).transpose(2, 3, 0, 1)
    return np.ascontiguousarray(out)


if __name__ == "__main__":
    data = np.load(sys.argv[1] if len(sys.argv) > 1 else "work/inputs.npz")
    out = kernel(**{k: data[k] for k in data.files})
    print("out", out.shape, out.dtype, float(np.abs(out).max()))
